# revision 1
# baseline (speedup 1.0000x reference)
"""Trainium2 Bass kernel for DeformableCrossAttentionModule.

Math (per batch b):
  offset = conv3x3(query, w_off) + b_off            # (18, H, W); ch 0:9 = dy, 9:18 = dx
  mod    = sigmoid(conv3x3(query, w_mod) + b_mod)   # (9, H, W)
  py/px  = base grid + kernel offset + offset       # (9, H, W)
  samp   = bilinear_sample(pad(value), px, py)      # (C, H, W, 9), zeros padding
  out    = einsum('chwn,ocn->bohw', samp * mod, w_out)

Sharding: 8 cores = (batch b in 0..3) x (row-half in 0..1); each core handles
32 output rows, streamed as 16 chunks of 128 positions (2 rows).

The axon-tunneled runtime rejects dynamic-offset DMA (indirect_dma_start /
dma_gather execute-fail), so the kernel runs in two device phases with the
bilinear x-pair gather — pure data movement — done on host between them:
  phase 1 (device): conv3x3 on PE, sampling coords / corner weights /
      flat indices on DVE -> idx + corner-weight tensors in DRAM
  host: fancy-index gather of (y, x0..x0+1) channel pairs (fp16)
  phase 2 (device): bilinear blend via diagonal-matrix matmuls on PE
      (PSUM-accumulated, also transposes to channel-major), then the
      1x1xN output projection as accumulating matmuls (fp16 in, fp32 acc)
"""

import sys

for _p in ("/opt/trn_rl_repo", "/opt/pypackages"):
    if _p not in sys.path:
        sys.path.insert(0, _p)

from contextlib import ExitStack

import numpy as np

import concourse.bacc as bacc
import concourse.bass as bass
import concourse.tile as tile
from concourse import mybir
from concourse.bass_utils import run_bass_kernel_spmd

F32 = mybir.dt.float32
F16 = mybir.dt.float16
I32 = mybir.dt.int32

B, C, H, W = 4, 256, 64, 64
N, PAD, OUTC = 9, 1, 256
Hp, Wp = H + 2 * PAD, W + 2 * PAD  # 66, 66
NCORES = 8
ROWS = H // 2          # output rows per core = 32
NCHUNK = ROWS // 2     # 16 chunks of 128 positions (2 rows x 64 cols)
ASCALE = float(Wp) / float(Wp - 1)  # 66/65, same for y since Hp == Wp
BIAS = 16.0            # keeps coords positive so trunc == floor


def _build_phase1():
    nc = bacc.Bacc("TRN2", target_bir_lowering=False, debug=False,
                   num_devices=NCORES)

    qs_d = nc.dram_tensor("qs", (3, 2, 128, H // 2 + 2, W), F32,
                          kind="ExternalInput").ap()
    wc_d = nc.dram_tensor("wc", (128, 9 * 2 * 27), F32,
                          kind="ExternalInput").ap()
    yb_d = nc.dram_tensor("ybase", (128, NCHUNK * N), F32,
                          kind="ExternalInput").ap()
    xb_d = nc.dram_tensor("xbase", (128, NCHUNK * N), F32,
                          kind="ExternalInput").ap()
    mb_d = nc.dram_tensor("mbias", (128, NCHUNK * N), F32,
                          kind="ExternalInput").ap()
    idx_d = nc.dram_tensor("idxo", (128, NCHUNK * 18), I32,
                           kind="ExternalOutput").ap()
    w4_d = nc.dram_tensor("w4o", (128, NCHUNK * 36), F16,
                          kind="ExternalOutput").ap()

    mult = mybir.AluOpType.mult
    add = mybir.AluOpType.add
    sub = mybir.AluOpType.subtract
    opmax = mybir.AluOpType.max
    opmin = mybir.AluOpType.min
    iseq = mybir.AluOpType.is_equal

    with tile.TileContext(nc) as tc, ExitStack() as ctx:
        cpool = ctx.enter_context(tc.tile_pool(name="const", bufs=1))
        wkpool = ctx.enter_context(tc.tile_pool(name="work", bufs=3))
        pcv = ctx.enter_context(tc.tile_pool(name="pconv", bufs=4,
                                             space="PSUM"))

        qtiles = {}
        for dx in range(3):
            for blk in range(2):
                qt = cpool.tile([128, 34 * W], F32, name=f"qs{dx}{blk}",
                                tag=f"qs{dx}{blk}")
                nc.sync.dma_start(qt[:], qs_d[dx, blk])
                qtiles[(dx, blk)] = qt
        wct = cpool.tile([128, 9 * 2 * 27], F32, tag="wc")
        nc.sync.dma_start(wct[:], wc_d[:])
        ybt = cpool.tile([128, NCHUNK * N], F32, tag="yb")
        nc.sync.dma_start(ybt[:], yb_d[:])
        xbt = cpool.tile([128, NCHUNK * N], F32, tag="xb")
        nc.sync.dma_start(xbt[:], xb_d[:])
        mbt = cpool.tile([128, NCHUNK * N], F32, tag="mb")
        nc.sync.dma_start(mbt[:], mb_d[:])

        for t in range(NCHUNK):
            pc = pcv.tile([128, 27], F32)
            for tap in range(9):
                dy, dx = divmod(tap, 3)
                for blk in range(2):
                    qo = (2 * t + dy) * W
                    lhsT = qtiles[(dx, blk)][:, qo: qo + 128]
                    co = (tap * 2 + blk) * 27
                    nc.tensor.matmul(
                        pc[:], lhsT=lhsT, rhs=wct[:, co: co + 27],
                        start=(tap == 0 and blk == 0),
                        stop=(tap == 8 and blk == 1),
                    )

            wk = wkpool.tile([128, 128], F32, tag="wk")

            def s(i):
                return wk[:, 9 * i: 9 * i + 9]

            cb9 = t * N
            oy, ox, ml = pc[:, 0:9], pc[:, 9:18], pc[:, 18:27]
            v = nc.vector
            v.scalar_tensor_tensor(s(0), oy, ASCALE, ybt[:, cb9: cb9 + 9],
                                   op0=mult, op1=add)
            v.scalar_tensor_tensor(s(1), ox, ASCALE, xbt[:, cb9: cb9 + 9],
                                   op0=mult, op1=add)
            v.tensor_tensor(s(13), ml, mbt[:, cb9: cb9 + 9], op=add)
            nc.scalar.activation(s(12), s(13),
                                 mybir.ActivationFunctionType.Sigmoid)
            # floor(y) robust to the cast rounding mode (trunc on sim, RNE
            # on hw): c = int(y); y0 = c - (c > y)
            flr = wkpool.tile([128, 18], I32, tag="flr")
            v.tensor_copy(out=flr[:, 0:9], in_=s(0))
            v.tensor_copy(out=flr[:, 9:18], in_=s(1))
            v.tensor_copy(out=s(4), in_=flr[:, 0:9])
            v.tensor_copy(out=s(5), in_=flr[:, 9:18])
            v.tensor_tensor(s(2), s(4), s(0), op=mybir.AluOpType.is_gt)
            v.tensor_tensor(s(3), s(5), s(1), op=mybir.AluOpType.is_gt)
            v.tensor_tensor(s(4), s(4), s(2), op=sub)        # y0 = floor
            v.tensor_tensor(s(5), s(5), s(3), op=sub)        # x0 = floor
            v.tensor_tensor(s(2), s(0), s(4), op=sub)        # fy
            v.tensor_tensor(s(3), s(1), s(5), op=sub)        # fx
            v.tensor_scalar(s(6), s(4), BIAS, BIAS + 64.0, op0=opmax,
                            op1=opmin)                        # y0c
            v.tensor_scalar(s(7), s(5), BIAS, BIAS + 64.0, op0=opmax,
                            op1=opmin)                        # x0c
            # row A = pixel y0c, row B = y0c+1; with d = y0c - y0:
            #   wA = [d==0]*(1-f) + [d==1]*f ;  wB = [d==0]*f + [d==-1]*(1-f)
            v.tensor_tensor(s(8), s(6), s(4), op=sub)         # d_y
            v.tensor_scalar(s(4), s(8), 0.0, None, op0=iseq)  # e0y
            v.tensor_scalar(s(10), s(8), 1.0, None, op0=iseq)   # e1y
            v.tensor_scalar(s(8), s(8), -1.0, None, op0=iseq)   # em1y
            v.tensor_scalar(s(13), s(2), -1.0, 1.0, op0=mult, op1=add)
            v.tensor_tensor(s(11), s(4), s(13), op=mult)
            v.tensor_tensor(s(10), s(10), s(2), op=mult)
            v.tensor_tensor(s(10), s(11), s(10), op=add)      # wyA
            v.tensor_tensor(s(11), s(4), s(2), op=mult)
            v.tensor_tensor(s(8), s(8), s(13), op=mult)
            v.tensor_tensor(s(2), s(11), s(8), op=add)        # wyB
            v.tensor_tensor(s(10), s(10), s(12), op=mult)     # wyA * mod
            v.tensor_tensor(s(2), s(2), s(12), op=mult)       # wyB * mod

            v.tensor_tensor(s(9), s(7), s(5), op=sub)         # d_x
            v.tensor_scalar(s(5), s(9), 0.0, None, op0=iseq)  # e0x
            v.tensor_scalar(s(11), s(9), 1.0, None, op0=iseq)   # e1x
            v.tensor_scalar(s(9), s(9), -1.0, None, op0=iseq)   # em1x
            v.tensor_scalar(s(13), s(3), -1.0, 1.0, op0=mult, op1=add)
            v.tensor_tensor(s(4), s(5), s(13), op=mult)
            v.tensor_tensor(s(11), s(11), s(3), op=mult)
            v.tensor_tensor(s(11), s(4), s(11), op=add)       # wxA
            v.tensor_tensor(s(4), s(5), s(3), op=mult)
            v.tensor_tensor(s(9), s(9), s(13), op=mult)
            v.tensor_tensor(s(3), s(4), s(9), op=add)         # wxB

            w4 = wkpool.tile([128, 36], F16, tag="w4")
            v.tensor_tensor(w4[:, 0:9], s(10), s(11), op=mult)    # A,pixA
            v.tensor_tensor(w4[:, 9:18], s(10), s(3), op=mult)    # A,pixB
            v.tensor_tensor(w4[:, 18:27], s(2), s(11), op=mult)   # B,pixA
            v.tensor_tensor(w4[:, 27:36], s(2), s(3), op=mult)    # B,pixB

            # flat gather indices: idx = (y0c-16)*66 + (x0c-16); row B = +66
            v.scalar_tensor_tensor(s(0), s(6), 66.0, s(7), op0=mult, op1=add)
            v.tensor_scalar(s(1), s(0), -(BIAS * 66.0 + BIAS), None, op0=add)
            v.tensor_scalar(s(3), s(1), 66.0, None, op0=add)
            idx32 = wkpool.tile([128, 18], I32, tag="idx")
            v.tensor_copy(out=idx32[:, 0:9], in_=s(1))
            v.tensor_copy(out=idx32[:, 9:18], in_=s(3))

            nc.sync.dma_start(idx_d[:, t * 18: (t + 1) * 18], idx32[:])
            nc.sync.dma_start(w4_d[:, t * 36: (t + 1) * 36], w4[:])

    nc.compile()
    return nc


def _build_phase2():
    nc = bacc.Bacc("TRN2", target_bir_lowering=False, debug=False,
                   num_devices=NCORES)

    g_d = nc.dram_tensor("gath", (NCHUNK, 128, 18 * 512), F16,
                         kind="ExternalInput").ap()
    w4_d = nc.dram_tensor("w4o", (128, NCHUNK * 36), F16,
                          kind="ExternalInput").ap()
    w2_d = nc.dram_tensor("w2", (128, N * 2 * 2 * 128), F16,
                          kind="ExternalInput").ap()
    id_d = nc.dram_tensor("ident", (128, 128), F16,
                          kind="ExternalInput").ap()
    out_d = nc.dram_tensor("out", (OUTC, ROWS, W), F32,
                           kind="ExternalOutput").ap()

    mult = mybir.AluOpType.mult

    with tile.TileContext(nc) as tc, ExitStack() as ctx:
        cpool = ctx.enter_context(tc.tile_pool(name="const", bufs=1))
        gpool = ctx.enter_context(tc.tile_pool(name="gath", bufs=3))
        dpool = ctx.enter_context(tc.tile_pool(name="diag", bufs=2))
        spool = ctx.enter_context(tc.tile_pool(name="samp", bufs=3))
        opool = ctx.enter_context(tc.tile_pool(name="ostg", bufs=2))
        psm = ctx.enter_context(tc.tile_pool(name="psamp", bufs=4,
                                             space="PSUM"))
        pout = ctx.enter_context(tc.tile_pool(name="pout", bufs=2,
                                              space="PSUM"))

        w2t = cpool.tile([128, N * 2 * 2 * 128], F16, tag="w2")
        nc.sync.dma_start(w2t[:], w2_d[:])
        w4t = cpool.tile([128, NCHUNK * 36], F16, tag="w4t")
        nc.sync.dma_start(w4t[:], w4_d[:])
        idt = cpool.tile([128, 128], F16, tag="id")
        nc.sync.dma_start(idt[:], id_d[:])

        for t in range(NCHUNK):
            gt = gpool.tile([128, 18 * 512], F16, tag="gt")
            nc.sync.dma_start(gt[:], g_d[t])
            gv = gt[:].rearrange("p (s e) -> p s e", e=512)

            # diag bank: bank[p, (k*9+n)*128 + f] = I[p, f] * w4[p, k*9+n]
            bank = dpool.tile([128, 36 * 128], F16, tag="bank")
            nc.vector.tensor_tensor(
                out=bank[:].rearrange("p (s f) -> p s f", f=128),
                in0=idt[:].rearrange("p (u f) -> p u f", u=1).to_broadcast(
                    [128, 36, 128]),
                in1=w4t[:, t * 36: (t + 1) * 36].rearrange(
                    "p (s u) -> p s u", u=1).to_broadcast([128, 36, 128]),
                op=mult,
            )

            po = [pout.tile([128, 128], F32, name=f"po{ob}", tag=f"po{ob}")
                  for ob in range(2)]
            for n in range(N):
                ps = psm.tile([128, 256], F32, tag="ps")
                for cb in range(2):
                    for r in range(2):
                        for pix in range(2):
                            k = r * 2 + pix
                            lo = pix * 256 + cb * 128
                            bo = (k * 9 + n) * 128
                            nc.tensor.matmul(
                                ps[:, cb * 128: cb * 128 + 128],
                                lhsT=gv[:, r * 9 + n, lo: lo + 128],
                                rhs=bank[:, bo: bo + 128],
                                start=(k == 0), stop=(k == 3),
                            )
                sampn = spool.tile([128, 256], F16, tag="sampn")
                nc.scalar.copy(sampn[:], ps[:])
                for cb in range(2):
                    for ob in range(2):
                        wo = ((n * 2 + cb) * 2 + ob) * 128
                        nc.tensor.matmul(
                            po[ob][:],
                            lhsT=w2t[:, wo: wo + 128],
                            rhs=sampn[:, cb * 128: cb * 128 + 128],
                            start=(n == 0 and cb == 0),
                            stop=(n == 8 and cb == 1),
                        )

            ost = opool.tile([128, 256], F32, tag="ost")
            nc.scalar.copy(ost[:, 0:128], po[0][:])
            nc.scalar.copy(ost[:, 128:256], po[1][:])
            for ob in range(2):
                nc.sync.dma_start(
                    out=out_d[ob * 128: ob * 128 + 128, 2 * t: 2 * t + 2, :],
                    in_=ost[:, ob * 128: ob * 128 + 128],
                )

    nc.compile()
    return nc


_CACHE = {}


def _get_programs():
    if "p1" not in _CACHE:
        _CACHE["p1"] = _build_phase1()
        _CACHE["p2"] = _build_phase2()
    return _CACHE["p1"], _CACHE["p2"]


def _host_prep(query, value, w_off, b_off, w_mod, b_mod, w_out):
    query = np.asarray(query, dtype=np.float32)
    value = np.asarray(value, dtype=np.float32)
    w_off = np.asarray(w_off, dtype=np.float32)
    b_off = np.asarray(b_off, dtype=np.float32)
    w_mod = np.asarray(w_mod, dtype=np.float32)
    b_mod = np.asarray(b_mod, dtype=np.float32)
    w_out = np.asarray(w_out, dtype=np.float32)

    qp = np.zeros((B, 2, 128, Hp, Wp), np.float32)
    qp[:, :, :, PAD:PAD + H, PAD:PAD + W] = query.reshape(B, 2, 128, H, W)
    qsx = np.stack([qp[:, :, :, :, dx: dx + W] for dx in range(3)], axis=1)

    vp = np.zeros((B, C, Hp, Wp), np.float32)
    vp[:, :, PAD:PAD + H, PAD:PAD + W] = value
    vcl = np.ascontiguousarray(
        vp.transpose(0, 2, 3, 1).reshape(B, Hp * Wp * C)).astype(np.float16)

    w27 = np.concatenate([w_off, w_mod], axis=0)
    wc = np.ascontiguousarray(
        w27.reshape(27, 2, 128, 9).transpose(2, 3, 1, 0)
    ).reshape(128, 9 * 2 * 27).astype(np.float32)

    w2 = np.ascontiguousarray(
        w_out.reshape(2, 128, 2, 128, N).transpose(3, 4, 2, 0, 1)
    ).reshape(128, N * 2 * 2 * 128).astype(np.float16)

    ident = np.eye(128, dtype=np.float16)

    n_ar = np.arange(N)
    pn_r = (n_ar // 3 - 1).astype(np.float32)
    pn_c = (n_ar % 3 - 1).astype(np.float32)
    p_ar = np.arange(128)
    row_in_chunk = (p_ar // W).astype(np.float32)
    col_in_chunk = (p_ar % W).astype(np.float32)
    t_ar = np.arange(NCHUNK, dtype=np.float32)

    xb = (ASCALE * (col_in_chunk[:, None, None] + pn_c[None, None, :]
                    + b_off[N:2 * N][None, None, :]) - 0.5 + BIAS)
    xb = np.broadcast_to(xb, (128, NCHUNK, N)).reshape(128, NCHUNK * N)
    xb = np.ascontiguousarray(xb, dtype=np.float32)
    mb = np.broadcast_to(b_mod[None, None, :], (128, NCHUNK, N))
    mb = np.ascontiguousarray(mb.reshape(128, NCHUNK * N), dtype=np.float32)

    in1, in2 = [], []
    for core in range(NCORES):
        b, half = divmod(core, 2)
        r0 = half * ROWS
        yb = (ASCALE * (r0 + 2.0 * t_ar[None, :, None]
                        + row_in_chunk[:, None, None] + pn_r[None, None, :]
                        + b_off[0:N][None, None, :]) - 0.5 + BIAS)
        yb = np.ascontiguousarray(
            yb.reshape(128, NCHUNK * N), dtype=np.float32)
        in1.append({
            "qs": np.ascontiguousarray(qsx[b, :, :, :, r0: r0 + 34, :]),
            "wc": wc,
            "ybase": yb,
            "xbase": xb,
            "mbias": mb,
        })
        in2.append({
            "w4o": None,  # filled after phase 1
            "gath": None,
            "w2": w2,
            "ident": ident,
        })
    return in1, in2, vcl


def kernel(**inputs):
    p1, p2 = _get_programs()
    in1, in2, vcl = _host_prep(**inputs)

    res1 = run_bass_kernel_spmd(p1, in1, core_ids=list(range(NCORES)))

    # host gather of bilinear x-pairs (pure data movement)
    off = np.arange(512)
    for core in range(NCORES):
        b = core // 2
        idx = res1.results[core]["idxo"].reshape(128, NCHUNK, 18)
        # gath[t, p, s*512:(s+1)*512] = vcl[b][idx[p,t,s]*256 : +512]
        gidx = (idx.transpose(1, 0, 2).reshape(NCHUNK, 128, 18, 1) * 256
                + off).reshape(NCHUNK, 128, 18 * 512)
        in2[core]["gath"] = vcl[b][gidx]
        in2[core]["w4o"] = res1.results[core]["w4o"]

    res2 = run_bass_kernel_spmd(p2, in2, core_ids=list(range(NCORES)))

    out = np.empty((B, OUTC, H, W), np.float32)
    for core in range(NCORES):
        b, half = divmod(core, 2)
        r0 = half * ROWS
        out[b, :, r0: r0 + ROWS, :] = res2.results[core]["out"]
    return out

